# revision 51
# baseline (speedup 1.0000x reference)
"""Cross-attention kernel for Trainium2, distributed over 8 NeuronCores.

Sharding: data-parallel over batch (4) x tensor-parallel over head groups (2).
Core c handles batch b = c//2, heads [4g, 4g+4) with g = c%2.

Mask-aware compaction (host-side): masked context positions contribute
exp(-50)~0 to softmax, so they are dropped before the kernel runs; masked
queries all produce the same uniform-attention row, computed host-side with
one mat-vec and scattered back. The device only processes unmasked queries
(padded to NWp, a multiple of 128) against [unmasked context | null]
(padded to W; null at fixed j = W-1 inside the padding, so the null
key/value DMAs never serialize against the projections). Per-core
variation (different mask counts per batch) lives entirely in the data —
zero-padded inputs plus the additive bias cmb — so all 8 cores run one
SPMD program.

All matmuls run in bf16 (4x the fp32 PE rate; fp32 accumulation in PSUM).
Per-core device pipeline:
  qT  = tanh(Wq_g^T @ x_c^T)                  [256, NWp] bf16 (d on parts)
  kT  = tanh(Wk_g^T @ ctx_c^T), null col W-1  [256, W]   bf16
  v   = ctx_c @ Wv_g (+ null row W-1)         [W, 4x64]  bf16 (j on parts)
  S^T = exp(0.125 * kT_h^T qT_h + cmb_j)      per (ic, jt, head-pair):
        scores head-per-PSUM-bank (sharing a bank between two matmuls'
        outputs fails on real HW), one Exp activation per 2-bank pair
  po   += S_tile^T(stationary) @ v_h          [128 i, 4h x 64] PSUM
        (v as the 64-wide moving operand: 8x fewer PE cycles than moving
        S); denominators via 1-column matmuls against ones into a
        dedicated bank. Two po accumulators pack per bank (2KB exactly),
        freeing a bank for the epilogue pool.
  divide by denominator, PE-transpose O -> OT [hd, i],
  out_partial = OT^T @ Wo_g                   [NWp, 512] bf16
Host sums the two head-group partials per batch (fp32), adds bo, scatters
into the full [2048, 512] output alongside the uniform masked-query row.

PSUM map: scores 2x2 banks (double-buffered pairs) + po 2 + den 1 +
epilogue/deferred-projection pool 1 = 8 banks.

Scheduling: Tanh and Exp share one activation table (`exp_and_others`), so
the remaining projection work (k chunks 1.., v tiles 4.., q chunks 1..)
and each chunk's epilogue (divide / transpose / out-projection) are
deferred into the attention stream via a deadline work-queue popped ~1
small thunk per (chunk, jt) unit; AV matmuls run one unit late so the
next unit's scores are always ready when an Exp retires. The Exp pipeline
(the bottleneck engine) starts as soon as the first x/ctx DMA chunks land
and never drains until the end.

PE instructions on TRN2 can carry at most ONE sync wait (walrus S3_LW /
ENGINE_NOP structs); Tile sometimes assigns more. `_split_pe_waits` runs
after scheduling and hoists extra waits onto PE nops inserted immediately
before the offending instruction.
"""

import ml_dtypes
import numpy as np

import concourse.bass as bass
import concourse.tile as tile
from concourse import bacc, bass_utils, mybir

FP = mybir.dt.float32
BF = mybir.dt.bfloat16
NPBF = np.dtype(ml_dtypes.bfloat16)
AF = mybir.ActivationFunctionType

B, N, M, DIM = 4, 2048, 2048, 512
HEADS, DH = 8, 64
G = 2          # head groups (tensor-parallel degree)
HG = 4         # heads per group
DG = HG * DH   # 256 dims per group
NEG = -50.0    # additive mask bias (exp(-50) ~ 2e-22)
SCALE = 1.0 / np.sqrt(DH)  # 0.125
VW = DH + 1    # v columns per head incl. ones column (den row)
ICW = 512      # i-chunk width (head-pair PSUM unit = 2 banks)

LAST_RESULTS = None
_CACHE = {}


def _chunks(total, width):
    out, c0 = [], 0
    while c0 < total:
        cw = min(width, total - c0)
        out.append((c0, cw))
        c0 += cw
    return out


def _build(nwp, w):
    nc = bacc.Bacc("TRN2", debug=False, num_devices=8, enable_partition_id=False)
    d = {}

    def inp(name, shape, dt=BF):
        d[name] = nc.dram_tensor(name, shape, dt, kind="ExternalInput").ap()

    inp("xT", [DIM, nwp])
    inp("cxT", [DIM, w])
    inp("wqk", [DIM, 2 * DG])  # wq | wk concatenated: one DMA, one latency
    inp("wv", [DIM, DG])
    inp("wo", [DG, DIM])
    inp("cmb", [128, w // 128], FP)  # bias: 0 attendable, NEG padding
    inp("nkt", [128, 1])             # tanh(null_key) tiled x2
    inp("nv", [1, DG])               # null_value tiled x4
    inp("ident", [128, 128])         # identity for PE transpose
    d["out"] = nc.dram_tensor("out", [nwp, DIM], BF, kind="ExternalOutput").ap()

    with tile.TileContext(nc) as tc:
        _body(tc, d, nwp, w)
    nc.compile()
    _split_pe_waits(nc)
    return nc


_SPLIT_SKIP = (
    "InstDrain", "InstUnconditionalBranch", "InstCall",
    "InstEventSemaphore", "InstRegisterMove", "InstDmaTrigger",
)


def _split_pe_waits(nc):
    """Hoist all-but-one sync waits from compute-engine instructions onto
    fresh same-engine nops placed immediately before them (TRN2 TPB
    instruction structs accept only one sync wait in walrus codegen)."""
    engines = {
        mybir.EngineType.PE: nc.tensor,
        mybir.EngineType.Activation: nc.scalar,
        mybir.EngineType.DVE: nc.vector,
        mybir.EngineType.Pool: nc.gpsimd,
        mybir.EngineType.SP: nc.sync,
    }
    total = 0
    for bb in nc.m.functions[0].blocks:
        new_insts = []
        for ins in bb.instructions:
            si = ins.sync_info
            eng = engines.get(getattr(ins, "engine", None))
            if (
                eng is not None
                and type(ins).__name__ not in _SPLIT_SKIP
                and si is not None
                and si.on_wait
                and len(si.on_wait) > 1
            ):
                waits = list(si.on_wait)
                for wt in waits[:-1]:
                    nop = eng._isa(
                        nc.isa.Opcode.NEURON_ISA_TPB_OPCODE_ENGINE_NOP,
                        {}, None, [], [], True,
                    )
                    nop.sync_info = mybir.SyncInfo(on_wait=[wt], on_update=[])
                    nc.inst_map[nop.name] = nop
                    new_insts.append(nop)
                    total += 1
                si.on_wait = waits[-1:]
            new_insts.append(ins)
        bb.instructions = new_insts
    return total


def _body(tc, d, nwp, w):
    nc = tc.nc
    jc = w // 128              # context j tiles
    ichunks = _chunks(nwp, ICW)
    kchunks = _chunks(w, 512)

    with (
        tc.tile_pool(name="consts", bufs=1) as consts,
        tc.tile_pool(name="big", bufs=1) as big,
        tc.tile_pool(name="spool", bufs=3) as spool,
        tc.tile_pool(name="small", bufs=4) as small,
        tc.tile_pool(name="sp", bufs=2, space="PSUM") as sp_ps,
        tc.tile_pool(name="po", bufs=2, space="PSUM") as po_ps,
        tc.tile_pool(name="dn", bufs=1, space="PSUM") as dn_ps,
        tc.tile_pool(name="ep", bufs=1, space="PSUM") as ep_ps,
    ):
        # ---- inputs; DMA order = first-use order ----
        wqk = consts.tile([128, 4, 2 * DG], BF)
        nc.sync.dma_start(wqk[:], d["wqk"].rearrange("(c p) d -> p c d", p=128))
        wq = wqk[:, :, 0:DG]
        wk = wqk[:, :, DG:2 * DG]

        xT = big.tile([128, 4, nwp], BF)
        xTd = d["xT"].rearrange("(c p) i -> p c i", p=128)
        cxT = big.tile([128, 4, w], BF)
        cxTd = d["cxT"].rearrange("(c p) j -> p c j", p=128)
        i0, cw0 = ichunks[0]
        nc.sync.dma_start(xT[:, :, 0:cw0], xTd[:, :, 0:cw0])
        k0, kw0 = kchunks[0]
        nc.sync.dma_start(cxT[:, :, 0:kw0], cxTd[:, :, 0:kw0])

        cmb = consts.tile([128, jc], FP)
        nc.sync.dma_start(cmb[:], d["cmb"])

        wv = consts.tile([128, 4, DG], BF)
        nc.sync.dma_start(wv[:], d["wv"].rearrange("(c p) d -> p c d", p=128))

        qT = big.tile([128, 2, nwp], BF)
        kT = big.tile([128, 2, w], BF)
        vsb = big.tile([128, jc, HG, DH], BF)
        OsbT = big.tile([128, 2, nwp], BF)
        ones_col = consts.tile([128, 1], BF)
        nc.vector.memset(ones_col[:], 1.0)

        # null key/value live at j = W-1 (inside the zero padding, so the
        # projections never write there and these DMAs stay off the
        # critical path)
        for dc in range(2):
            nc.sync.dma_start(kT[:, dc, w - 1:w], d["nkt"])

        for c0, cw in kchunks[1:]:
            nc.sync.dma_start(cxT[:, :, c0:c0 + cw], cxTd[:, :, c0:c0 + cw])
        for c0, cw in ichunks[1:]:
            nc.sync.dma_start(xT[:, :, c0:c0 + cw], xTd[:, :, c0:c0 + cw])

        wo = consts.tile([128, 2, DIM], BF)
        nc.sync.dma_start(wo[:], d["wo"].rearrange("(c p) o -> p c o", p=128))
        ident = consts.tile([128, 128], BF)
        nc.sync.dma_start(ident[:], d["ident"])

        # ---- projection emitters (PSUM from the sp ring: its consumers
        # never depend on future work, so sharing cannot deadlock) ----
        def emit_qproj_dc(n, dc, pool=None):
            pool = pool or sp_ps
            c0, cw = ichunks[n]
            ps = pool.tile([128, 512], FP,
                           tag="sp" if pool is sp_ps else "ep",
                           name=f"psq{dc}{n}")
            for cc in range(4):
                nc.tensor.matmul(
                    ps[:, 0:cw],
                    wq[:, cc, dc * 128:(dc + 1) * 128],
                    xT[:, cc, c0:c0 + cw],
                    start=(cc == 0), stop=(cc == 3),
                )
            nc.scalar.activation(qT[:, dc, c0:c0 + cw], ps[:, 0:cw], AF.Tanh)

        def emit_qproj(n):
            for dc in range(2):
                emit_qproj_dc(n, dc)

        def emit_kproj_dc(n, dc, pool=None):
            pool = pool or sp_ps
            c0, cw = kchunks[n]
            e = 1 if n == len(kchunks) - 1 else 0  # col w-1 = null key
            ps = pool.tile([128, 512], FP,
                           tag="sp" if pool is sp_ps else "ep",
                           name=f"psk{dc}{n}")
            for cc in range(4):
                nc.tensor.matmul(
                    ps[:, 0:cw],
                    wk[:, cc, dc * 128:(dc + 1) * 128],
                    cxT[:, cc, c0:c0 + cw],
                    start=(cc == 0), stop=(cc == 3),
                )
            nc.scalar.activation(kT[:, dc, c0:c0 + cw - e],
                                 ps[:, 0:cw - e], AF.Tanh)

        def emit_kproj(n):
            for dc in range(2):
                emit_kproj_dc(n, dc)

        def emit_vproj(jt):
            # prologue v tiles use the (still empty) po ring; queued ones
            # use the ep pool so the score ring never waits behind them
            pool = po_ps if jt < 4 else ep_ps
            ps = pool.tile([128, DG], FP, tag=pool is po_ps and "po" or "ep",
                           name=f"psv{jt}")
            for cc in range(4):
                nc.tensor.matmul(
                    ps[:],
                    cxT[:, cc, jt * 128:(jt + 1) * 128],
                    wv[:, cc, :],
                    start=(cc == 0), stop=(cc == 3),
                )
            nc.vector.tensor_copy(
                vsb[:, jt, :, :],
                ps[:].rearrange("p (h e) -> p h e", h=HG),
            )
            if jt == jc - 1:  # overwrite row w-1 (null token value)
                nc.sync.dma_start(vsb[127:128, jt, :, :],
                                  d["nv"].rearrange("a (h e) -> a h e", h=HG))

        # ---- epilogue emitters (divide / transpose / out-projection) ----
        def emit_div(pic, po, den, nsub):
            Ods = []
            for isub in range(nsub):
                rden = small.tile([128, HG], FP, tag="rdn", name=f"rd{pic}{isub}")
                nc.vector.reciprocal(rden[:], den[:, isub * HG:(isub + 1) * HG])
                Od = small.tile([128, HG, DH], BF, tag="od", name=f"od{pic}{isub}")
                nc.vector.tensor_mul(Od[:], po[isub // 2][:, isub % 2, :, :],
                                     rden[:].to_broadcast((128, HG, DH)))
                Ods.append(Od)
            return Ods

        def emit_transpose(pic, Ods, base, isub, hp, last):
            # O [i, hd] -> OT [hd, i]; each transpose gets its own PSUM
            # zero region (transpose start=True zeroes the whole region;
            # shared-bank matmul output tricks fail on real HW). The ep
            # pool keeps these out of the score ring.
            it0 = base + isub * 128
            pT = ep_ps.tile([128, 128], BF, tag="ep",
                            name=f"pT{pic}{isub}{hp}")
            nc.tensor.transpose(
                pT[:],
                Ods[isub][:, 2 * hp:2 * hp + 2, :]
                .rearrange("p h e -> p (h e)"),
                ident[:],
            )
            if last and isub % 2 == hp:
                nc.scalar.copy(OsbT[:, hp, it0:it0 + 128], pT[:])
            else:
                nc.vector.tensor_copy(OsbT[:, hp, it0:it0 + 128], pT[:])

        def emit_outproj(pic, base, isub, last):
            it0 = base + isub * 128
            pf = ep_ps.tile([128, DIM], FP, tag="ep", name=f"pf{pic}{isub}")
            for dc in range(2):
                nc.tensor.matmul(
                    pf[:],
                    OsbT[:, dc, it0:it0 + 128],
                    wo[:, dc, :],
                    start=(dc == 0), stop=(dc == 1),
                )
            fo = spool.tile([128, DIM], BF, tag="fo", name=f"fo{pic}{isub}")
            if last and isub % 2 == 0:
                nc.scalar.copy(fo[:], pf[:])
            else:
                nc.vector.tensor_copy(fo[:], pf[:])
            nc.sync.dma_start(d["out"][it0:it0 + 128, :], fo[:])

        # ---- prologue: just enough to start the exp stream ----
        emit_qproj(0)
        emit_kproj(0)
        for jt in range(min(4, jc)):
            emit_vproj(jt)

        # Deferred work (remaining projections + per-chunk epilogues) runs
        # from a deadline queue: after each (chunk, jt) unit we pop thunks
        # until ~700ns of PE work is emitted, or earlier-deadline thunks
        # force out. Keeps every insertion small so the Exp stream (the
        # bottleneck) never waits behind a big PE lump or a stale sp-ring
        # slot.
        queue = []  # (deadline unit, seq, pe_cost_ns, fn)
        seq_n = [0]

        def push(deadline, cost, fn, *a):
            queue.append((deadline, seq_n[0], cost, fn, a))
            seq_n[0] += 1

        for n in range(1, len(kchunks)):
            for dc in range(2):
                push(max(0, 4 * n - 3 + dc), 900, emit_kproj_dc, n, dc, ep_ps)
        for jt in range(4, jc):
            push(max(0, jt - 2), 450, emit_vproj, jt)
        for n in range(1, len(ichunks)):
            for dc in range(2):
                push(max(0, n * jc - 4 + dc), 900, emit_qproj_dc, n, dc, ep_ps)
        queue.sort(key=lambda t: (t[0], t[1]))

        def pop_work(u):
            spent = 0
            while queue and (queue[0][0] <= u or spent < 400):
                _, _, cost, fn, a = queue.pop(0)
                fn(*a)
                spent += cost

        # ---- flash attention over i chunks ----
        nunits = [0]
        prev = None  # (chunk idx, po tiles, base, nsub) pending epilogue
        for ici, (ic0, icw) in enumerate(ichunks):
            nsub = icw // 128
            if prev is not None:
                pic, ppo, pbase, pnsub, pden = prev
                pOds = emit_div(pic, ppo, pden, pnsub)
                for isub in range(pnsub):
                    for hp in range(2):
                        push(10 ** 9, 300, emit_transpose, pic, pOds, pbase,
                             isub, hp, False)
                for isub in range(pnsub):
                    push(10 ** 9, 600, emit_outproj, pic, pbase, isub, False)
            # two po accumulators pack per PSUM bank ([128, 2, 4, 64] f32
            # = exactly 2KB); denominators accumulate in their own bank
            # via 1-column matmuls against a ones vector (~free on PE)
            po = [po_ps.tile([128, 2, HG, DH], FP, tag="po",
                             name=f"po{ici}{s}") for s in range((nsub + 1) // 2)]
            den = dn_ps.tile([128, 4 * HG], FP, tag="dn", name=f"dn{ici}")

            def emit_av(Ssbs, jt, nsub=nsub, po=po, den=den):
                # isub-major: at jt 0 the matmul into po[isub] waits on
                # the previous chunk's divide freeing that bank, so the
                # latest-gated isubs should come last
                for isub in range(nsub):
                    for hp in range(2):
                        for hh in range(2):
                            h = 2 * hp + hh
                            S_sl = Ssbs[hp][:, hh, isub * 128:(isub + 1) * 128]
                            nc.tensor.matmul(
                                po[isub // 2][:, isub % 2, h, :],
                                S_sl,
                                vsb[:, jt, h, :],
                                start=(jt == 0 and h == 0 and isub % 2 == 0),
                                stop=(jt == jc - 1 and h == HG - 1
                                      and (isub % 2 == 1 or isub == nsub - 1)),
                            )
                            nc.tensor.matmul(
                                den[:, isub * HG + h:isub * HG + h + 1],
                                S_sl,
                                ones_col[:],
                                start=(jt == 0 and h == 0 and isub == 0),
                                stop=(jt == jc - 1 and h == HG - 1
                                      and isub == nsub - 1),
                            )

            pend_av = None  # previous unit's (Ssbs, jt): AV runs one unit
            for jt in range(jc):  # late so scores never wait behind it
                Ssbs = []
                for hp in range(2):
                    # head-per-bank score pairs: each matmul fills exactly
                    # one PSUM bank with its own start/stop group (shared-
                    # bank flag tricks fail on real HW)
                    sps = sp_ps.tile([128, 2, 512], FP, tag="sp",
                                     name=f"sp{ici}{jt}{hp}")
                    for hh in range(2):
                        h = 2 * hp + hh
                        prow = DH * (h % 2)
                        nc.tensor.matmul(
                            sps[:, hh, 0:icw],
                            kT[prow:prow + DH, h // 2, jt * 128:(jt + 1) * 128],
                            qT[prow:prow + DH, h // 2, ic0:ic0 + icw],
                            start=True, stop=True,
                        )
                    Ssb = spool.tile([128, 2, 512], BF, tag="s",
                                     name=f"s{ici}{jt}{hp}")
                    nc.scalar.activation(Ssb[:, :, 0:icw], sps[:, :, 0:icw],
                                         AF.Exp, bias=cmb[:, jt:jt + 1],
                                         scale=float(SCALE))
                    Ssbs.append(Ssb)
                if pend_av is not None:
                    emit_av(*pend_av)
                pend_av = (Ssbs, jt)
                pop_work(nunits[0])
                nunits[0] += 1
            emit_av(*pend_av)
            prev = (ici, po, ic0, nsub, den)
        # drain the queue, then the final epilogue (nothing left to overlap
        # with: split its copies between ACT and DVE)
        while queue:
            _, _, _, fn, a = queue.pop(0)
            fn(*a)
        pic, ppo, pbase, pnsub, pden = prev
        Ods = emit_div(pic, ppo, pden, pnsub)
        for isub in range(pnsub):
            for hp in range(2):
                emit_transpose(pic, Ods, pbase, isub, hp, True)
        for isub in range(pnsub):
            emit_outproj(pic, pbase, isub, True)


def _plan(mask, context_mask):
    qidx = [np.nonzero(mask[b])[0] for b in range(B)]
    cidx = [np.nonzero(context_mask[b])[0] for b in range(B)]
    nmax = max(1, max(len(q) for q in qidx))
    mmax = max(len(c) for c in cidx) + 1  # + null at j=0
    nwp = -(-nmax // 128) * 128
    w = -(-mmax // 128) * 128
    return qidx, cidx, nwp, w


def _core_inputs(inputs, core, qidx, cidx, nwp, w):
    b, g = core // 2, core % 2
    x = np.asarray(inputs["x"], np.float32)
    context = np.asarray(inputs["context"], np.float32)
    Wq = np.asarray(inputs["Wq"], np.float32)
    Wkv = np.asarray(inputs["Wkv"], np.float32)
    Wo = np.asarray(inputs["Wo"], np.float32)
    null_key = np.asarray(inputs["null_key"], np.float32)
    null_value = np.asarray(inputs["null_value"], np.float32)

    nb, mb = len(qidx[b]), len(cidx[b])
    xc = np.zeros((nwp, DIM), np.float32)
    xc[:nb] = x[b][qidx[b]]
    cxc = np.zeros((w, DIM), np.float32)
    cxc[:mb] = context[b][cidx[b]]  # col w-1 = null token
    cmb = np.full(w, NEG, np.float32)
    cmb[:mb] = 0.0
    cmb[w - 1] = 0.0
    gs = slice(g * DG, (g + 1) * DG)
    return {
        "xT": np.ascontiguousarray(xc.T).astype(NPBF),
        "cxT": np.ascontiguousarray(cxc.T).astype(NPBF),
        "wqk": np.ascontiguousarray(
            np.concatenate([Wq[:, gs], Wkv[:, gs]], axis=1)).astype(NPBF),
        "wv": np.ascontiguousarray(
            Wkv[:, DIM + g * DG: DIM + (g + 1) * DG]).astype(NPBF),
        "wo": np.ascontiguousarray(Wo[gs, :]).astype(NPBF),
        "cmb": np.ascontiguousarray(cmb.reshape(w // 128, 128).T),
        "nkt": np.tanh(np.tile(null_key, 2)).reshape(128, 1).astype(NPBF),
        "nv": np.tile(null_value, HG).reshape(1, DG).astype(NPBF),
        "ident": np.eye(128, dtype=np.float32).astype(NPBF),
    }


def kernel(x, context, mask, context_mask, Wq, Wkv, Wo, bo, null_key, null_value):
    global LAST_RESULTS
    inputs = {
        "x": x, "context": context, "mask": mask, "context_mask": context_mask,
        "Wq": Wq, "Wkv": Wkv, "Wo": Wo, "bo": bo,
        "null_key": null_key, "null_value": null_value,
    }
    mask = np.asarray(mask)
    context_mask = np.asarray(context_mask)
    qidx, cidx, nwp, w = _plan(mask, context_mask)
    key = (nwp, w)
    if key not in _CACHE:
        _CACHE[key] = _build(nwp, w)
    nc = _CACHE[key]
    in_maps = [_core_inputs(inputs, core, qidx, cidx, nwp, w)
               for core in range(8)]
    res = bass_utils.run_bass_kernel_spmd(nc, in_maps, core_ids=list(range(8)))
    LAST_RESULTS = res

    Wkv_np = np.asarray(Wkv, np.float32)
    Wo_np = np.asarray(Wo, np.float32)
    bo_np = np.asarray(bo, np.float32)
    nv_full = np.tile(np.asarray(null_value, np.float32), HEADS)
    ctx_np = np.asarray(context, np.float32)
    out = np.empty((B, N, DIM), np.float32)
    for b in range(B):
        nb = len(qidx[b])
        dev = (res.results[2 * b]["out"].astype(np.float32)
               + res.results[2 * b + 1]["out"].astype(np.float32))
        # uniform attention row for masked queries: mean over ALL 2049
        # (null + full context) values, projected through Wo
        v_sum = ctx_np[b].sum(0) @ Wkv_np[:, DIM:] + nv_full
        uniform = (v_sum / (M + 1)) @ Wo_np + bo_np
        out[b] = uniform
        if nb:
            out[b][qidx[b]] = dev[:nb] + bo_np
    return out
